# revision 1
# baseline (speedup 1.0000x reference)
"""Windowed multi-head self-attention Bass kernel for Trainium2.

Shapes (hardcoded): input [64, 256, 1536] fp32 (packed qkv, 32 heads x 16 dim),
rel_bias_table [127, 32] fp32. Output [64, 256, 512] fp32.

Sharding: data-parallel over the window axis B=64 across 8 NeuronCores
(8 windows per core). The tiny bias table is preprocessed on host into
per-head streaming blocks (float32r-rounded) and replicated to every core.

Per-core algorithm, processed in window PAIRS (wp = 2 windows):
  - Load input rows as [128, 1536] SBUF tiles (2 n-chunks x 2 windows).
  - PE-transpose q/k sections in [128,128] blocks; VectorE copies PSUM
    results into qt/kt tiles casting to float32r (TF32-like PE format:
    1 cycle/row vs fp32's 4).
  - Stitched K=128 matmul operands:
      KD[quad]: rows 0-63 = kT of 4 heads (1 DMA per window-pair),
                rows 64-127 = one-hot U_mch (static; m-chunk in free dim).
      QS[head]: rows 0-63 = zeros + qT_h at 16*(h%4) (1 DMA per pair),
                rows 64-127 = bias rows G'_h (static).
    One K=128 float32r matmul per (head, m-chunk) computes
    S^T = (q.k)^T + rel-pos bias straight into PSUM: the one-hot rows
    inject the bias, the zero rows mask the other heads' qT.
  - exp on ScalarE from PSUM in [128,1024] two-head batches -> bf16 P^T;
    1/sqrt(32) folded into the activation scale (no max-subtraction:
    |scores| <= ~5, fp32 exp is safe).
  - PV in bf16: lhsT = v' [128, 17] (v + ones column -> row 16 = softmax
    denominator), rhs = P^T chunks, col-tiled 4 heads per PSUM tile.
  - PE-transpose back to [n, c] layout, reciprocal + broadcast multiply
    on VectorE, contiguous output DMA.

DMA descriptor-generation is the main non-compute cost; DMAs are split
between the SP HWDGE path and the gpsimd SWDGE path.
"""

import numpy as np
from contextlib import ExitStack

import concourse.bass as bass
import concourse.bacc as bacc
import concourse.tile as tile
from concourse import mybir
from concourse.bass_utils import run_bass_kernel_spmd

F32 = mybir.dt.float32
F32R = mybir.dt.float32r
BF16 = mybir.dt.bfloat16

NCORES = 8
B = 64
W = B // NCORES
N = 256
C = 1536
NH = 32
HD = 16
SCALE = float(NH) ** -0.5


def _build_kernel_body(ctx, tc, out, inp, gbias, uhot, ident):
    nc = tc.nc

    singles = ctx.enter_context(tc.tile_pool(name="singles", bufs=1))
    inpool = ctx.enter_context(tc.tile_pool(name="inpool", bufs=2))
    tpool = ctx.enter_context(tc.tile_pool(name="tpool", bufs=2))
    vpool = ctx.enter_context(tc.tile_pool(name="vpool", bufs=2))
    ppool = ctx.enter_context(tc.tile_pool(name="ppool", bufs=4))
    opool = ctx.enter_context(tc.tile_pool(name="opool", bufs=3))
    rpool = ctx.enter_context(tc.tile_pool(name="rpool", bufs=6))
    ps_s = ctx.enter_context(tc.tile_pool(name="ps_s", bufs=2, space="PSUM"))
    ps_pv = ctx.enter_context(tc.tile_pool(name="ps_pv", bufs=2, space="PSUM"))
    ps_tr = ctx.enter_context(tc.tile_pool(name="ps_tr", bufs=2, space="PSUM"))

    # --- static tiles ---
    id_t = singles.tile([128, 128], F32, tag="ident")
    nc.sync.dma_start(out=id_t[:], in_=ident)

    # KD[hq]: [128, w2, mch2, 128]; rows 0-63 kT quad (dynamic),
    # rows 64-127 U_mch (static, w-replicated).
    KD = [singles.tile([128, 2, 2, 128], F32R, tag=f"kd{q}", name=f"kd{q}")
          for q in range(8)]
    # QS[h]: [128, w2, 256]; rows 0-63 zeros + qT_h at 16*(h%4) (dynamic),
    # rows 64-127 G'_h (static, w-replicated).
    QS = [singles.tile([128, 2, 256], F32R, tag=f"qs{h}", name=f"qs{h}")
          for h in range(NH)]
    NP = W // 2
    st = {}

    def emit_load(wp):
        xin = {}
        for ws in range(2):
            for ch in range(2):
                t = inpool.tile([128, C], F32, tag=f"xin{ws}{ch}",
                                name=f"xin{ws}{ch}")
                nc.sync.dma_start(
                    out=t[:],
                    in_=inp[2 * wp + ws, ch * 128:(ch + 1) * 128, :])
                xin[(ws, ch)] = t
        st[wp] = {"xin": xin}

    def emit_transposes(wp):
        xin = st[wp]["xin"]
        qt = tpool.tile([128, 4, 2, 256], F32R, tag="qt", name="qt")
        kt = tpool.tile([128, 4, 2, 256], F32R, tag="kt", name="kt")
        for cb in range(4):
            for sec, dst in ((0, qt), (512, kt)):
                for ch in range(2):
                    tr = ps_tr.tile([128, 256], F32, tag="tr", name="tr")
                    for ws in range(2):
                        nc.tensor.transpose(
                            tr[:, ws * 128:(ws + 1) * 128],
                            xin[(ws, ch)][:, sec + cb * 128: sec + (cb + 1) * 128],
                            id_t[:],
                        )
                    nc.vector.tensor_copy(
                        dst[:, cb, :, ch * 128:(ch + 1) * 128],
                        tr.rearrange("p (w n) -> p w n", w=2),
                    )
        vb = {}
        for ws in range(2):
            for ch in range(2):
                t = vpool.tile([128, NH, 17], BF16, tag=f"vb{ws}{ch}",
                               name=f"vb{ws}{ch}")
                nc.gpsimd.tensor_copy(
                    t[:, :, 0:16],
                    xin[(ws, ch)][:, 1024:1536].rearrange(
                        "p (h d) -> p h d", d=16),
                )
                if wp < 2:
                    nc.gpsimd.memset(t[:, :, 16:17], 1.0)
                vb[(ws, ch)] = t
        st[wp]["qt"] = qt
        st[wp]["kt"] = kt
        st[wp]["vb"] = vb

    def emit_stitch_quad(wp, hq):
        qt, kt = st[wp]["qt"], st[wp]["kt"]
        cb = hq // 2
        row = 64 * (hq % 2)
        nc.gpsimd.dma_start(
            out=KD[hq][0:64, :, :, :],
            in_=kt[row:row + 64, cb, :, :].rearrange(
                "p w (m n) -> p w m n", m=2))
        for t in range(4):
            h = 4 * hq + t
            hh = h % 8
            dst = QS[h][16 * t:16 * t + 16, :, :]
            src = qt[16 * hh:16 * hh + 16, cb, :, :]
            if t < 3:
                nc.sync.dma_start(out=dst, in_=src)
            else:
                nc.gpsimd.dma_start(out=dst, in_=src)

    def emit_stitch(wp):
        for hq in range(8):
            emit_stitch_quad(wp, hq)

    def emit_init_quad(hq):
        u_dst = bass.AP(tensor=KD[hq].tensor,
                        offset=KD[hq].offset + 64 * 512,
                        ap=[[512, 64], [256, 2], [1, 256]])
        u_src = bass.AP(tensor=uhot.tensor, offset=uhot.offset,
                        ap=[[256, 64], [0, 2], [1, 256]])
        nc.gpsimd.dma_start(out=u_dst, in_=u_src)
        for t in range(4):
            h = 4 * hq + t
            nc.vector.memset(QS[h][0:64, :, :].bitcast(F32), 0.0)
            g_dst = bass.AP(tensor=QS[h].tensor,
                            offset=QS[h].offset + 64 * 512,
                            ap=[[512, 64], [256, 2], [1, 256]])
            g_src = bass.AP(tensor=gbias.tensor,
                            offset=gbias.offset + h * 64 * 256,
                            ap=[[256, 64], [0, 2], [1, 256]])
            if h % 2 == 0:
                nc.sync.dma_start(out=g_dst, in_=g_src)
            else:
                nc.gpsimd.dma_start(out=g_dst, in_=g_src)

    def emit_compute(wp, ws, stitch_next=False):
        vb = st[wp]["vb"]
        w = 2 * wp + ws
        oacc = []
        for ch in range(2):
            oacc.append(opool.tile([128, 512], F32, tag=f"oacc{ch}",
                                   name=f"oacc{ch}"))

        for hg in range(8):
            ppair = []
            for sub in range(2):
                ps = ps_s.tile([128, 1024], F32, tag="scores",
                               name="scores")
                for par in range(2):
                    h = 4 * hg + 2 * sub + par
                    for mch in range(2):
                        qq = 2 * par + mch
                        nc.tensor.matmul(
                            ps[:, qq * 256:(qq + 1) * 256],
                            lhsT=KD[hg][:, ws, mch, :],
                            rhs=QS[h][:, ws, :],
                            start=True,
                            stop=True,
                        )
                pt = ppool.tile([128, 1024], BF16, tag="pt", name="pt")
                nc.scalar.activation(
                    pt[:], ps[:], mybir.ActivationFunctionType.Exp,
                    scale=SCALE,
                )
                ppair.append(pt)

            pv = ps_pv.tile([128, 256], F32, tag="pv", name="pv")
            for j in range(4):
                sub, par = divmod(j, 2)
                h = 4 * hg + j
                pt = ppair[sub]
                for mch in range(2):
                    qq = 2 * par + mch
                    nc.tensor.matmul(
                        pv[32 * j:32 * j + 17, :],
                        lhsT=vb[(ws, mch)][:, h, :],
                        rhs=pt[:, qq * 256:(qq + 1) * 256],
                        start=(mch == 0),
                        stop=(mch == 1),
                        tile_position=(0, 32 * j),
                    )

            pvs = opool.tile([128, 256], F32, tag="pvs", name="pvs")
            nc.vector.tensor_copy(pvs[:], pv[:])
            for ch in range(2):
                tro = ps_tr.tile([128, 128], F32, tag="tr", name="tre")
                nc.tensor.transpose(tro[:],
                                    pvs[:, ch * 128:(ch + 1) * 128],
                                    id_t[:])
                trv = tro.rearrange("p (j x) -> p j x", x=32)
                rcp = rpool.tile([128, 4, 1], F32, tag="rcp", name="rcp")
                nc.vector.reciprocal(rcp[:], trv[:, :, 16:17])
                rb = rcp[:]
                rbcast = bass.AP(
                    tensor=rb.tensor,
                    offset=rb.offset,
                    ap=[rb.ap[0], rb.ap[1], [0, 16]],
                )
                nc.vector.tensor_mul(
                    oacc[ch][:, 64 * hg:64 * hg + 64].rearrange(
                        "p (j d) -> p j d", d=16),
                    trv[:, :, 0:16],
                    rbcast,
                )
            if stitch_next:
                emit_stitch_quad(wp + 1, hg)

        for ch in range(2):
            nc.sync.dma_start(
                out=out[w, ch * 128:(ch + 1) * 128, :], in_=oacc[ch][:]
            )

    # software-pipelined pair loop: next pair's load/transpose/stitch are
    # emitted inside the current pair's compute so DGE + PE-transpose work
    # overlaps QK/exp/PV instead of stalling at pair boundaries.
    emit_load(0)
    emit_transposes(0)
    for hq in range(8):
        emit_init_quad(hq)
        emit_stitch_quad(0, hq)
    for wp in range(NP):
        if wp + 1 < NP:
            emit_load(wp + 1)
        emit_compute(wp, 0)
        if wp + 1 < NP:
            emit_transposes(wp + 1)
        emit_compute(wp, 1, stitch_next=(wp + 1 < NP))
        st.pop(wp - 1, None)


def build_nc():
    nc = bacc.Bacc(
        "TRN2", target_bir_lowering=False, debug=False, num_devices=NCORES
    )
    inp = nc.dram_tensor("inp", [W, N, C], F32, kind="ExternalInput").ap()
    gbias = nc.dram_tensor("gbias", [NH, 64, N], F32R,
                           kind="ExternalInput").ap()
    uhot = nc.dram_tensor("uhot", [64, 2, 128], F32R,
                          kind="ExternalInput").ap()
    ident = nc.dram_tensor("ident", [128, 128], F32, kind="ExternalInput").ap()
    out = nc.dram_tensor("out", [W, N, NH * HD], F32,
                         kind="ExternalOutput").ap()
    with tile.TileContext(nc) as tc:
        with ExitStack() as ctx:
            _build_kernel_body(ctx, tc, out, inp, gbias, uhot, ident)
    nc.compile()
    return nc


def _round_f32r(x):
    b = np.ascontiguousarray(x).view(np.uint32)
    lsb = (b >> 12) & 1
    return ((b + 0x7FF + lsb) & 0xFFFFF000).view(np.float32)


def _host_consts(table):
    # G'[h, i, n] = table[n//4 - i + 63, h] * sqrt(32), i in [0, 64)
    j = np.arange(N) // 4
    i0 = np.arange(64)
    idx = j[None, :] - i0[:, None] + 63  # [64, 256], values in [0, 126]
    g = table[idx]  # [64, 256, NH]
    gbias = np.ascontiguousarray(np.transpose(g, (2, 0, 1))) * np.float32(
        1.0 / SCALE
    )
    gbias = _round_f32r(gbias.astype(np.float32))
    # U_mch[i, c, m] = 1 if (m//4 + 32c) == i
    m4 = np.arange(128) // 4
    uhot = (m4[None, None, :] + 32 * np.arange(2)[None, :, None]
            == np.arange(64)[:, None, None]).astype(np.float32)
    ident = np.eye(128, dtype=np.float32)
    return gbias, np.ascontiguousarray(uhot), ident


_NC_CACHE = None


def kernel(input, rel_bias_table):
    global _NC_CACHE
    x = np.ascontiguousarray(np.asarray(input, dtype=np.float32))
    tbl = np.asarray(rel_bias_table, dtype=np.float32)
    assert x.shape == (B, N, C), x.shape
    assert tbl.shape == (127, NH), tbl.shape

    if _NC_CACHE is None:
        _NC_CACHE = build_nc()
    nc = _NC_CACHE

    gbias, uhot, ident = _host_consts(tbl)
    in_maps = [
        {
            "inp": np.ascontiguousarray(x[i * W:(i + 1) * W]),
            "gbias": gbias,
            "uhot": uhot,
            "ident": ident,
        }
        for i in range(NCORES)
    ]
    res = run_bass_kernel_spmd(nc, in_maps, list(range(NCORES)))
    return np.concatenate([res.results[i]["out"] for i in range(NCORES)], axis=0)

